# revision 7
# baseline (speedup 1.0000x reference)
"""LocalVarianceMap Trainium2 kernel.

Computes var = box7x7(mean_c(x)^2) - box7x7(mean_c(x))^2 ... precisely:
  lum  = mean over channel of x            (B,1,H,W)
  mean = 7x7 'same' box mean of lum
  sqm  = 7x7 'same' box mean of lum^2
  out  = sqm - mean^2

Full input x: (16, 3, 1024, 1024) fp32. Data-parallel over batch:
8 NeuronCores x 2 images each.

Per-core pipeline (per 128-row tile, row-major layout: partition=h, free=w):
  1. DMA 3 channel row-blocks into one SBUF tile
  2. lum = x0+x1+x2            (GPSIMD tensor adds, zero-padded cols)
  3. sq  = lum^2               (ACT Square)
  4. h1/h2 = horizontal sliding 7-sum of lum/sq via tensor_tensor_scan
     (state = (new + state) - old7)  -- one DVE op per path
  5. vertical 7-sum of both paths with one banded fp32 matmul group on PE
     (K = input rows incl. 3-row halo held in-tile, M = output rows)
  6. mean^2 = ACT Square(S1 * 1/49);  var = (S2 * 1/49) - mean^2 (DVE stt)
  7. DMA out
Tiles overlap by 6 input rows so each output tile's vertical halo lives
inside one SBUF tile (single matmul group, K<=128).
"""

import sys

if "/opt/trn_rl_repo" not in sys.path:
    sys.path.insert(0, "/opt/trn_rl_repo")

import numpy as np
from contextlib import ExitStack

import concourse.bass as bass
import concourse.bacc as bacc
import concourse.tile as tile
from concourse import mybir

H = 1024
W = 1024
C = 3
PER_CORE_B = 2
N_CORES = 8
K7 = 7
PADL, PADR = 7, 3
LW = PADL + W + PADR      # padded lum/sq width (1034)
SCAN_N = W + 3            # scan output cols; h[:, j+3] = centered 7-sum at col j

# Vertical tiling: tile 0 outputs rows 0..124 from input rows 0..127;
# tiles 1..7 output 122 rows each from 128 overlapping input rows;
# tile 8 outputs rows 979..1023 from input rows 976..1023.
def _tiles():
    specs = []
    specs.append(dict(r0=0, nr=128, K=128, M=125, out_r0=0, w=0))
    for t in range(1, 8):
        specs.append(dict(r0=122 * t, nr=128, K=128, M=122, out_r0=122 * t + 3, w=1))
    specs.append(dict(r0=976, nr=48, K=48, M=45, out_r0=979, w=2))
    assert specs[-1]["out_r0"] + specs[-1]["M"] == H
    return specs


def band_weights() -> np.ndarray:
    """Three banded [128,128] blocks side by side: W0 | Wmid | Wlast."""
    wb = np.zeros((128, 3 * 128), np.float32)
    # tile 0: out row m (0..124) <- input rows max(m-3,0)..m+3
    for m in range(125):
        for k in range(max(m - 3, 0), m + 4):
            wb[k, m] = 1.0
    # mid tiles: out row (r0+3+m) <- input rows r0+m..r0+m+6
    for m in range(122):
        for k in range(m, m + 7):
            wb[k, 128 + m] = 1.0
    # last tile: K=48, out row 979+m (m<45) <- input rows 976+m..min(...,1023)
    for m in range(45):
        for k in range(m, min(m + 7, 48)):
            wb[k, 256 + m] = 1.0
    return wb


def build_nc(finalize: bool = True) -> bass.Bass:
    nc = bacc.Bacc("TRN2", target_bir_lowering=False)
    dt = mybir.dt.float32
    x = nc.dram_tensor("x", [PER_CORE_B, C, H, W], dt, kind="ExternalInput")
    wbt = nc.dram_tensor("wb", [128, 3 * 128], dt, kind="ExternalInput")
    y = nc.dram_tensor("y", [PER_CORE_B, 1, H, W], dt, kind="ExternalOutput")

    # lum tile holds 3*lum (channel sum); S1 = 147*mean, S2 = 441*sqm
    inv147 = float(np.float32(1.0) / np.float32(147.0))
    inv441 = float(np.float32(1.0) / np.float32(441.0))

    with tile.TileContext(nc) as tc, ExitStack() as ctx:
        cpool = ctx.enter_context(tc.tile_pool(name="const", bufs=1))
        xpool = ctx.enter_context(tc.tile_pool(name="xin", bufs=3))
        lpool = ctx.enter_context(tc.tile_pool(name="lum", bufs=2))
        spool = ctx.enter_context(tc.tile_pool(name="sq", bufs=2))
        hpool = ctx.enter_context(tc.tile_pool(name="hsum", bufs=2))
        mpool = ctx.enter_context(tc.tile_pool(name="m2", bufs=2))
        vpool = ctx.enter_context(tc.tile_pool(name="vout", bufs=3))
        ppool = ctx.enter_context(tc.tile_pool(name="psum", bufs=2, space="PSUM"))

        WB = cpool.tile([128, 3 * 128], dt)
        nc.sync.dma_start(out=WB[:], in_=wbt[:, :])

        for b in range(PER_CORE_B):
            for sp in _tiles():
                r0, nr, K, M, out_r0, wsel = (
                    sp["r0"], sp["nr"], sp["K"], sp["M"], sp["out_r0"], sp["w"],
                )
                X = xpool.tile([128, C * W], dt, tag="X")
                for c in range(C):
                    nc.sync.dma_start(
                        out=X[0:nr, c * W : (c + 1) * W],
                        in_=x[b, c, r0 : r0 + nr, :],
                    )

                lum = lpool.tile([128, LW], dt, tag="lum")
                nc.vector.memset(lum[0:nr, 0:PADL], 0.0)
                nc.vector.memset(lum[0:nr, PADL + W : LW], 0.0)
                nc.gpsimd.tensor_add(
                    lum[0:nr, PADL : PADL + W], X[0:nr, 0:W], X[0:nr, W : 2 * W]
                )
                nc.gpsimd.tensor_add(
                    lum[0:nr, PADL : PADL + W],
                    lum[0:nr, PADL : PADL + W],
                    X[0:nr, 2 * W : 3 * W],
                )

                sq = spool.tile([128, LW], dt, tag="sq")
                nc.gpsimd.memset(sq[0:nr, 0:PADL], 0.0)
                nc.gpsimd.memset(sq[0:nr, PADL + W : LW], 0.0)
                nc.scalar.activation(
                    sq[0:nr, PADL : PADL + W],
                    lum[0:nr, PADL : PADL + W],
                    mybir.ActivationFunctionType.Square,
                )

                h1 = hpool.tile([128, SCAN_N], dt, tag="h1")
                h2 = hpool.tile([128, SCAN_N], dt, tag="h2")
                nc.vector.tensor_tensor_scan(
                    out=h1[0:nr, :],
                    data0=lum[0:nr, PADL : PADL + SCAN_N],
                    data1=lum[0:nr, 0:SCAN_N],
                    initial=0.0,
                    op0=mybir.AluOpType.add,
                    op1=mybir.AluOpType.subtract,
                )
                nc.vector.tensor_tensor_scan(
                    out=h2[0:nr, :],
                    data0=sq[0:nr, PADL : PADL + SCAN_N],
                    data1=sq[0:nr, 0:SCAN_N],
                    initial=0.0,
                    op0=mybir.AluOpType.add,
                    op1=mybir.AluOpType.subtract,
                )

                S = ppool.tile([128, 2 * W], dt, tag="S")
                lhsT = WB[0:K, 128 * wsel : 128 * wsel + M]
                for cnk in range(2):
                    nc.tensor.matmul(
                        S[0:M, 512 * cnk : 512 * (cnk + 1)],
                        lhsT,
                        h1[0:K, 3 + 512 * cnk : 3 + 512 * (cnk + 1)],
                        start=True,
                        stop=True,
                    )
                for cnk in range(2):
                    nc.tensor.matmul(
                        S[0:M, W + 512 * cnk : W + 512 * (cnk + 1)],
                        lhsT,
                        h2[0:K, 3 + 512 * cnk : 3 + 512 * (cnk + 1)],
                        start=True,
                        stop=True,
                    )

                m2 = mpool.tile([128, W], dt, tag="m2")
                nc.scalar.activation(
                    m2[0:M, :],
                    S[0:M, 0:W],
                    mybir.ActivationFunctionType.Square,
                    scale=inv147,
                )

                V = vpool.tile([128, W], dt, tag="V")
                nc.vector.scalar_tensor_tensor(
                    out=V[0:M, :],
                    in0=S[0:M, W : 2 * W],
                    scalar=inv441,
                    in1=m2[0:M, :],
                    op0=mybir.AluOpType.mult,
                    op1=mybir.AluOpType.subtract,
                )

                nc.sync.dma_start(out=y[b, 0, out_r0 : out_r0 + M, :], in_=V[0:M, :])

    if finalize:
        nc.finalize()
    return nc


def kernel(x, kernel_size):
    assert int(kernel_size) == K7
    x = np.ascontiguousarray(np.asarray(x, dtype=np.float32))
    B = x.shape[0]
    assert x.shape == (B, C, H, W) and B == PER_CORE_B * N_CORES

    from concourse.bass_utils import run_bass_kernel_spmd

    nc = build_nc()
    wb = band_weights()
    in_maps = [
        {"x": x[i * PER_CORE_B : (i + 1) * PER_CORE_B], "wb": wb}
        for i in range(N_CORES)
    ]
    res = run_bass_kernel_spmd(nc, in_maps, list(range(N_CORES)))
    y = np.concatenate([res.results[i]["y"] for i in range(N_CORES)], axis=0)
    return y
